# revision 4
# baseline (speedup 1.0000x reference)
"""Trainium2 kernel for nn_AmharicHNet300M (ragged_sequence).

Strategy (data parallel, 8 NeuronCores):
  - The dominant FLOPs (detector MLP layers, 145 GFLOP fp32) run on the 8
    NeuronCores via a tiled Bass/Tile GEMM kernel, sharded by rows (pure DP,
    weights replicated) — fp32 PE matmuls because the boundary decision
    `final > 0.5` has a minimum margin of 2.9e-6 and flipped boundaries change
    the output discretely.
  - Nonlinearities of the boundary path (erf-GELU / sigmoid) are applied in
    float64 on the host between the two device GEMM phases, so no LUT
    approximation error can flip a boundary bit.
  - The remaining stages (cosine chunker glue, block-diagonal attention,
    segment pooling, chunk FFN + LayerNorm) follow the reference numerics.
"""

import os
import sys

for _p in ("/opt/trn_rl_repo", "/root/.axon_site/_ro/trn_rl_repo"):
    if os.path.isdir(_p) and _p not in sys.path:
        sys.path.insert(0, _p)

import numpy as np

# ---- problem constants (hardcoded per spec) ----
B, S, D = 4, 1024, 1536
H, HD = 12, 128
MAXC, MAXLEN = 256, 1024
THRESH = 0.5
NCORES = 8

_GRAPH_CACHE = {}


def _erf(v):
    try:
        from scipy.special import erf
        return erf(v)
    except Exception:  # pragma: no cover - vectorized erf fallback
        import math
        return np.vectorize(math.erf)(v)


def _gelu64(v):
    v = v.astype(np.float64)
    return 0.5 * v * (1.0 + _erf(v / np.sqrt(2.0)))


def _build_gemm(name, K, M, N, nb):
    """Per-core graph: out[i] = a_i.T @ b_i for i in range(nb).

    a_i: [K, M] (kxm, pre-transposed on host), b_i: [K, N] (kxn).
    K % 128 == 0, M % 128 == 0, N % 128 == 0. Output [nb, M, N] fp32.
    N is tiled in chunks of <=512 that divide N.
    """
    import concourse.bass as bass
    import concourse.mybir as mybir
    from concourse import bacc, tile

    f32 = mybir.dt.float32
    nc = bacc.Bacc("TRN2", target_bir_lowering=False, debug=False,
                   num_devices=NCORES)

    a_exts = [nc.declare_dram_parameter(f"a{i}", [K, M], f32, isOutput=False)
              for i in range(nb)]
    b_exts = [nc.declare_dram_parameter(f"b{i}", [K, N], f32, isOutput=False)
              for i in range(nb)]
    out_ext = nc.declare_dram_parameter("out", [nb, M, N], f32, isOutput=True)

    NT = 512
    while N % NT:
        NT //= 2
    kt, mt, nt = K // 128, M // 128, N // NT

    with tile.TileContext(nc) as tc:
        with (
            tc.tile_pool(name="apool", bufs=2) as apool,
            tc.tile_pool(name="bpool", bufs=2) as bpool,
            tc.tile_pool(name="opool", bufs=4) as opool,
            tc.tile_pool(name="psum", bufs=4, space=bass.MemorySpace.PSUM) as ppool,
        ):
            for i in range(nb):
                # resident A slab for this batch entry: kt tiles of [128, M]
                a_tiles = []
                for k in range(kt):
                    t = apool.tile([128, M], f32, tag=f"a{k}")
                    nc.sync.dma_start(t[:], a_exts[i][k * 128:(k + 1) * 128, :])
                    a_tiles.append(t)
                for n in range(nt):
                    b_tiles = []
                    for k in range(kt):
                        t = bpool.tile([128, NT], f32, tag=f"b{k}")
                        nc.sync.dma_start(
                            t[:], b_exts[i][k * 128:(k + 1) * 128,
                                            n * NT:(n + 1) * NT])
                        b_tiles.append(t)
                    for m in range(mt):
                        ps = ppool.tile([128, NT], f32)
                        for k in range(kt):
                            nc.tensor.matmul(
                                ps[:],
                                a_tiles[k][:, m * 128:(m + 1) * 128],
                                b_tiles[k][:],
                                start=(k == 0), stop=(k == kt - 1))
                        ot = opool.tile([128, NT], f32)
                        nc.vector.tensor_copy(ot[:], ps[:])
                        nc.sync.dma_start(
                            out_ext[i, m * 128:(m + 1) * 128,
                                    n * NT:(n + 1) * NT], ot[:])
    nc.compile()
    return nc


def _gemm_spmd(name, a_shards, b_shards):
    """Run out = a.T @ b per core. a_shards: list[NCORES] of list[nb] of [K,M];
    b_shards likewise [K,N]. Returns list[NCORES] of [nb, M, N]."""
    from concourse.bass_utils import run_bass_kernel_spmd

    nb = len(a_shards[0])
    K, M = a_shards[0][0].shape
    N = b_shards[0][0].shape[1]
    key = (name, K, M, N, nb)
    if key not in _GRAPH_CACHE:
        _GRAPH_CACHE[key] = _build_gemm(name, K, M, N, nb)
    nc = _GRAPH_CACHE[key]

    in_maps = []
    for c in range(NCORES):
        m = {}
        for i in range(nb):
            m[f"a{i}"] = np.ascontiguousarray(a_shards[c][i], dtype=np.float32)
            m[f"b{i}"] = np.ascontiguousarray(b_shards[c][i], dtype=np.float32)
        in_maps.append(m)
    res = run_bass_kernel_spmd(nc, in_maps, core_ids=list(range(NCORES)))
    return [r["out"] for r in res.results]


def _interp1d64(y, L_out):
    L_in = y.shape[1]
    src = np.clip((np.arange(L_out, dtype=np.float64) + 0.5) * (L_in / L_out)
                  - 0.5, 0.0, L_in - 1)
    i0 = np.floor(src).astype(np.int64)
    i1 = np.minimum(i0 + 1, L_in - 1)
    w = src - i0
    return y[:, i0] * (1.0 - w) + y[:, i1] * w


def kernel(x, Wp, bp, detW1, detb1, detW2, detb2, detW3, detb3,
           in_proj_w, in_proj_b, out_w, out_b, size_emb, pos_enc,
           procW1, procb1, procW2, procb2, ln_g, ln_b):
    x = np.asarray(x, dtype=np.float32)

    # ---------- device phase 1: h1_pre = bi @ concat_n(detW1[n].T) ----------
    # bi: [B*(S-1), 2D] -> padded to 4096 rows; sharded 512 rows/core (kxm).
    bi = np.concatenate([x[:, :-1], x[:, 1:]], axis=-1).reshape(B * (S - 1),
                                                               2 * D)
    rows = B * (S - 1)                      # 4092
    rows_pad = NCORES * 512                 # 4096
    biT = np.zeros((2 * D, rows_pad), np.float32)
    biT[:, :rows] = bi.T
    W1T_all = np.ascontiguousarray(
        np.transpose(np.asarray(detW1, np.float32), (2, 0, 1)).reshape(
            2 * D, 3 * D))                  # [k, n*d]: col n*D+d = detW1[n,d,k]

    a_sh = [[np.ascontiguousarray(biT[:, c * 512:(c + 1) * 512])]
            for c in range(NCORES)]
    b_sh = [[W1T_all]] * NCORES
    outs1 = _gemm_spmd("h1", a_sh, b_sh)
    h1_pre = np.concatenate([o[0] for o in outs1], axis=0)[:rows]  # [4092, 3D]
    h1_pre = h1_pre.reshape(rows, 3, D).transpose(1, 0, 2)          # [3, 4092, D]

    # host: exact erf-gelu in f64
    h1 = _gelu64(h1_pre + np.asarray(detb1, np.float64)[:, None, :])

    # ---------- device phase 2: h2_pre[n] = h1[n] @ detW2[n].T ----------
    a_sh2, b_sh2 = [], []
    W2T = [np.ascontiguousarray(np.asarray(detW2[n], np.float32).T)
           for n in range(3)]               # [D, D//2]
    h1T = [np.zeros((D, rows_pad), np.float32) for _ in range(3)]
    for n in range(3):
        h1T[n][:, :rows] = h1[n].astype(np.float32).T
    for c in range(NCORES):
        a_sh2.append([np.ascontiguousarray(h1T[n][:, c * 512:(c + 1) * 512])
                      for n in range(3)])
        b_sh2.append(W2T)
    outs2 = _gemm_spmd("h2", a_sh2, b_sh2)
    h2_pre = np.concatenate(outs2, axis=1)[:, :rows]     # [3, 4092, D//2]

    h2 = _gelu64(h2_pre + np.asarray(detb2, np.float64)[:, None, :])
    logits = np.einsum('nsh,nh->ns', h2, np.asarray(detW3, np.float64)) \
        + np.asarray(detb3, np.float64)[:, None]
    learned = 1.0 / (1.0 + np.exp(-logits))              # [3, 4092]
    avg_learned = learned.mean(axis=0).reshape(B, S - 1)

    # ---------- boundary base path (host, f64 glue on f32 x_ling) ----------
    x_ling = (x.reshape(B * S, D) @ np.asarray(Wp, np.float32).T
              + np.asarray(bp, np.float32)).reshape(B, S, D).astype(np.float64)
    sims = []
    for scale in (1, 2, 4):
        xs = x_ling[:, ::scale]
        a, b2 = xs[:, :-1], xs[:, 1:]
        na = np.maximum(np.linalg.norm(a, axis=-1), 1e-8)
        nb_ = np.maximum(np.linalg.norm(b2, axis=-1), 1e-8)
        cs = np.sum(a * b2, axis=-1) / (na * nb_)
        sims.append(_interp1d64(cs, S - 1))
    avg_sim = np.mean(np.stack(sims, 0), axis=0)
    base = 0.5 * (1.0 - avg_sim)
    final = 0.6 * base + 0.4 * avg_learned               # [B, S-1] f64

    # ---------- segments ----------
    bits = np.concatenate([np.ones((B, 1), bool), final > THRESH], axis=1)
    seg = np.cumsum(bits.astype(np.int64), axis=1) - 1   # [B, S]

    # ---------- attention (block-diagonal by segment) ----------
    xf = x.reshape(B * S, D)
    qkv = (xf @ np.asarray(in_proj_w, np.float32).T
           + np.asarray(in_proj_b, np.float32)).reshape(B, S, 3, H, HD)
    q = np.ascontiguousarray(qkv[:, :, 0])               # [B, S, H, HD]
    k = np.ascontiguousarray(qkv[:, :, 1])
    v = np.ascontiguousarray(qkv[:, :, 2])
    scale = np.float32(1.0 / np.sqrt(HD))
    attn_out = np.empty((B, S, D), np.float32)
    ow = np.asarray(out_w, np.float32)
    for bix in range(B):
        sc = np.einsum('qhd,khd->hqk', q[bix], k[bix],
                       optimize=True).astype(np.float32) * scale
        same = seg[bix][None, :, None] == seg[bix][None, None, :]
        sc = np.where(same, sc, np.float32(-1e9))
        sc -= sc.max(axis=-1, keepdims=True)
        np.exp(sc, out=sc)
        sc /= sc.sum(axis=-1, keepdims=True)
        ctx = np.einsum('hqk,khd->qhd', sc, v[bix],
                        optimize=True).reshape(S, D).astype(np.float32)
        attn_out[bix] = ctx @ ow.T + np.asarray(out_b, np.float32)

    # ---------- segment mean pooling ----------
    se = np.asarray(size_emb, np.float32)
    pe = np.asarray(pos_enc, np.float32)[0]
    chunk = np.zeros((B, MAXC, D), np.float32)
    for bix in range(B):
        sums = np.zeros((MAXC, D), np.float64)
        segb = seg[bix]
        msk = segb < MAXC
        np.add.at(sums, segb[msk], attn_out[bix][msk].astype(np.float64))
        counts = np.bincount(segb[msk], minlength=MAXC).astype(np.float64)
        mean = (sums / np.maximum(counts, 1.0)[:, None]).astype(np.float32)
        clen = np.minimum(counts.astype(np.int64), MAXLEN - 1)
        ch = mean + se[clen]
        ch[counts == 0] = 0.0
        chunk[bix] = ch + pe

    # ---------- chunk processor ----------
    cf = chunk.reshape(B * MAXC, D)
    h = cf @ np.asarray(procW1, np.float32).T + np.asarray(procb1, np.float32)
    h = _gelu64(h).astype(np.float32)
    y = h @ np.asarray(procW2, np.float32).T + np.asarray(procb2, np.float32)
    mu = y.mean(axis=-1, keepdims=True)
    var = y.var(axis=-1, keepdims=True)
    y = ((y - mu) / np.sqrt(var + 1e-5) * np.asarray(ln_g, np.float32)
         + np.asarray(ln_b, np.float32))
    return y.reshape(B, MAXC, D).astype(np.float32)
